# revision 30
# baseline (speedup 1.0000x reference)
"""Multi-head causal self-attention (B=2, S=2048, D=1024, H=16) on 8 TRN2 NeuronCores.

Sharding: data-parallel over batch (2) x tensor-parallel over heads (4 groups of
4 heads). Each core computes Q/K/V projections for its 4 heads, causal
flash-style attention (scores kept transposed [k, q] so no on-chip transposes
are needed), and a partial output projection against its row-slice of W_O.
Host sums the 4 partials per batch and adds the output bias.

All matmul operands are bf16. Scores accumulate in fp32 PSUM; exp runs on ACT
into bf16 P tiles, restricted to the causally-valid column spans. Softmax
denominators come from an extra all-ones column appended to V (the P@V matmul
also produces the row sums). The normalization chain is pipelined across
blocks: the denominator copies + one [1,1024] reciprocal (DVE) + one combined
64-partition GpSimd broadcast run in the next block's first filler slot, and
the normalize multiplies (reading P@V straight out of PSUM) in its second
slot — DVE never bursts at block boundaries.

Schedule: attention q-block groups are software-pipelined (QK of group g+1
issues before P@V of group g) and projection / output-projection chunks are
interleaved as per-group PE fillers. DMAs are trigger-light (each dma_start
costs ~600ns of sequencer time) and priority-ordered; the two prologue
projections' c-loops are interleaved so the PE starts as soon as the first
x/wq chunks land and stays busy through the DMA-paced phase. Tail
out-projection tiles borrow the retired score-tile PSUM banks and split their
contraction so the PE works underneath the final normalization chain.
"""

import contextlib
import sys

import numpy as np

sys.path.insert(0, "/opt/trn_rl_repo")

import concourse.bass as bass  # noqa: E402
import concourse.tile as tile  # noqa: E402
from concourse import bacc, mybir  # noqa: E402
from concourse.bass_utils import run_bass_kernel_spmd  # noqa: E402

from ml_dtypes import bfloat16  # noqa: E402

F32 = mybir.dt.float32
BF16 = mybir.dt.bfloat16
AF = mybir.ActivationFunctionType

B, S, D, H = 2, 2048, 1024, 16
DH = D // H          # 64
TPG = 4              # tensor-parallel groups
HPC = H // TPG       # 4 heads per core
CH = HPC * DH        # 256 channels per core
CHA = CH + HPC       # 260: V channels augmented with a ones column per head
NEG = -1.0e9
N_CORES = 8

NQ = S // 512    # 4 q-blocks of 512
NT = S // 128    # 16 s-tiles / k-blocks

_PROG = None  # cached compiled Bass program


def _build_program():
    nc = bacc.Bacc("TRN2", target_bir_lowering=False, debug=False,
                   num_devices=N_CORES)

    xT = nc.dram_tensor("xT", [D, S], BF16, kind="ExternalInput").ap()
    wq = nc.dram_tensor("wq", [D, CH], BF16, kind="ExternalInput").ap()
    wk = nc.dram_tensor("wk", [D, CH], BF16, kind="ExternalInput").ap()
    wv = nc.dram_tensor("wv", [D, CHA], BF16, kind="ExternalInput").ap()
    wo = nc.dram_tensor("wo", [CH, D], BF16, kind="ExternalInput").ap()
    bq = nc.dram_tensor("bq", [128, 2], F32, kind="ExternalInput").ap()
    bk = nc.dram_tensor("bk", [128, 2], F32, kind="ExternalInput").ap()
    bv = nc.dram_tensor("bv", [1, CHA], BF16, kind="ExternalInput").ap()
    tri = nc.dram_tensor("tri", [128, 128], F32, kind="ExternalInput").ap()
    onesf = nc.dram_tensor("onesf", [1, 64], F32, kind="ExternalInput").ap()
    out = nc.dram_tensor("out", [S, D], BF16, kind="ExternalOutput").ap()

    with tile.TileContext(nc) as tc, contextlib.ExitStack() as ctx:
        const = ctx.enter_context(tc.tile_pool(name="const", bufs=1))
        qt = const.tile([128, 2, S], BF16)     # Q^T/8 (+bq/8): chunk m = heads 2m,2m+1
        kt = const.tile([128, 2, S], BF16)     # K^T (+bk)
        va = const.tile([128, NT, CHA], BF16)  # V augmented: [s, head-major 65-col blocks]
        otn = const.tile([128, 2, S], BF16)    # normalized attention out, transposed
        tri_t = const.tile([128, 128], F32)
        ones64f = const.tile([1, 64], F32)
        bq_t = const.tile([128, 2], F32)
        bk_t = const.tile([128, 2], F32)
        bv_row = const.tile([1, CHA], BF16)
        bvb = const.tile([128, CHA], BF16)
        wo_t = const.tile([128, 2, D], BF16)
        xt = const.tile([128, 8, S], BF16)
        wq_t = const.tile([128, 8, CH], BF16)
        wk_t = const.tile([128, 8, CH], BF16)
        wv_t = const.tile([128, 8, CHA], BF16)
        xTr = xT.rearrange("(a p) s -> p a s", p=128)
        xTc = xT.rearrange("(a p) s -> a p s", p=128)
        wqr = wq.rearrange("(a p) c -> p a c", p=128)
        wqc = wq.rearrange("(a p) c -> a p c", p=128)
        wkr = wk.rearrange("(a p) c -> p a c", p=128)
        wkc = wk.rearrange("(a p) c -> a p c", p=128)
        wvr = wv.rearrange("(a p) c -> p a c", p=128)
        wor = wo.rearrange("(a p) n -> p a n", p=128)

        # trigger-light priority-ordered DMA (each dma_start costs ~600ns of
        # sequencer time, so keep the count low): x cols 0:512 per-c (for
        # prologue pipelining) interleaved across both queues, m0 projection
        # weights, then wv + consts, then the x remainder in q-block order,
        # then the late-needed weights (m1 projections, W_O)
        # small first chunk: descriptor generation for the big strided wq
        # DMA takes ~1.3us, which would delay the very first matmul
        nc.scalar.dma_start(wq_t[:, 0, 0:128], wqc[0][:, 0:128])
        nc.scalar.dma_start(wq_t[:, 1:8, 0:128], wqr[:, 1:8, 0:128])
        for c in range(0, 8, 2):
            nc.sync.dma_start(xt[:, c, 0:512], xTc[c][:, 0:512])
            nc.scalar.dma_start(xt[:, c + 1, 0:512], xTc[c + 1][:, 0:512])
        nc.sync.dma_start(wk_t[:, :, 0:128], wkr[:, :, 0:128])
        nc.sync.dma_start(wv_t, wvr)
        nc.scalar.dma_start(bq_t, bq)
        nc.scalar.dma_start(bk_t, bk)
        nc.scalar.dma_start(bv_row, bv)
        nc.scalar.dma_start(tri_t, tri)
        nc.scalar.dma_start(ones64f, onesf)
        # V bias row replicated across partitions on GpSimd (off-path; same
        # DSP program as the softmax broadcasts, so no library churn)
        nc.gpsimd.partition_broadcast(bvb, bv_row)
        # n=1 split in two half-triggers so the n=1 projections' c-loops can
        # start on the first half ~1.5us earlier
        nc.sync.dma_start(xt[:, 0:4, 512:1024], xTr[:, 0:4, 512:1024])
        nc.sync.dma_start(xt[:, 4:8, 512:1024], xTr[:, 4:8, 512:1024])
        for n in range(2, NQ):
            nc.sync.dma_start(xt[:, :, n * 512:(n + 1) * 512],
                              xTr[:, :, n * 512:(n + 1) * 512])
        nc.scalar.dma_start(wq_t[:, :, 128:256], wqr[:, :, 128:256])
        nc.scalar.dma_start(wk_t[:, :, 128:256], wkr[:, :, 128:256])
        nc.sync.dma_start(wo_t, wor)

        # preload the ACT exp table set while ACT is otherwise idle
        nc.scalar.activation(ones64f, ones64f, AF.Exp)

        with tc.tile_pool(name="sm", bufs=4) as sm, \
             tc.tile_pool(name="psp", bufs=2, space="PSUM") as psp:

            def qk_proj(w_t, dst, bias_t, m, n):
                # one 512-col n-chunk of the Q/K projection; DVE evacuation
                # (bias is a per-partition [128,1] tensor-scalar operand)
                ps = psp.tile([128, 512], F32, tag="ps", name="ps")
                for c in range(8):
                    nc.tensor.matmul(
                        ps, (w_t[:, c, m * 128:(m + 1) * 128]),
                        (xt[:, c, n * 512:(n + 1) * 512]),
                        start=(c == 0), stop=(c == 7))
                nc.vector.tensor_scalar_add(
                    dst[:, m, n * 512:(n + 1) * 512], ps, bias_t[:, m:m + 1])

            def qk_proj00():
                # prologue: Q and K n=0 chunks with interleaved c-loops into
                # two PSUM banks, so the PE tracks the per-chunk DMA arrivals
                psq = psp.tile([128, 512], F32, tag="ps", name="ps")
                psk = psp.tile([128, 512], F32, tag="ps", name="ps")
                for c in range(8):
                    nc.tensor.matmul(psq, (wq_t[:, c, 0:128]),
                                     (xt[:, c, 0:512]),
                                     start=(c == 0), stop=(c == 7))
                    nc.tensor.matmul(psk, (wk_t[:, c, 0:128]),
                                     (xt[:, c, 0:512]),
                                     start=(c == 0), stop=(c == 7))
                nc.vector.tensor_scalar_add(qt[:, 0, 0:512], psq,
                                            bq_t[:, 0:1])
                nc.vector.tensor_scalar_add(kt[:, 0, 0:512], psk,
                                            bk_t[:, 0:1])

            def v_proj(t):
                # V (not transposed): stationary = x^T tile, moving = wv_aug;
                # bias (with the ones columns) folds into the DVE evacuation
                ps = psp.tile([128, 512], F32, tag="ps", name="ps")
                psv = ps[:, 0:CHA]
                for c in range(8):
                    nc.tensor.matmul(
                        psv, (xt[:, c, t * 128:(t + 1) * 128]),
                        (wv_t[:, c, :]), start=(c == 0), stop=(c == 7))
                nc.vector.tensor_add(va[:, t, :], psv, bvb)

            def out_tile(jj, t, ns=(0, 1)):
                # 512-col n-chunks of one 128-row tile of the partial output
                # projection
                so = sm.tile([128, 1024], BF16, tag="so", name="so", bufs=3)
                for n in ns:
                    ps = psp.tile([128, 512], F32, tag="ps", name="ops")
                    for c2 in range(2):
                        nc.tensor.matmul(
                            ps, (otn[:, c2, t * 128:(t + 1) * 128]),
                            (wo_t[:, c2, n * 512:(n + 1) * 512]),
                            start=(c2 == 0), stop=(c2 == 1))
                    nc.vector.tensor_copy(so[:, n * 512:(n + 1) * 512], ps)
                    nc.sync.dma_start(
                        out[t * 128:(t + 1) * 128, n * 512:(n + 1) * 512],
                        so[:, n * 512:(n + 1) * 512])

            def out_tile_tail(t, pst):
                # tail out-proj tile in a retired score-tile PSUM bank pair
                # ([128,1024]); c2=0 was issued before the final norm chain
                for n in range(2):
                    nc.tensor.matmul(
                        pst[:, n * 512:(n + 1) * 512],
                        (otn[:, 1, t * 128:(t + 1) * 128]),
                        (wo_t[:, 1, n * 512:(n + 1) * 512]),
                        start=False, stop=(n == 1))
                so = sm.tile([128, 1024], BF16, tag="so", name="so", bufs=3)
                nc.scalar.copy(so, pst)
                nc.sync.dma_start(out[t * 128:(t + 1) * 128, :], so)

            def out_tail_c0(t):
                pst = psp.tile([128, 1024], F32, tag="st", name="ost")
                for n in range(2):
                    nc.tensor.matmul(
                        pst[:, n * 512:(n + 1) * 512],
                        (otn[:, 0, t * 128:(t + 1) * 128]),
                        (wo_t[:, 0, n * 512:(n + 1) * 512]),
                        start=True, stop=False)
                return pst

            # two-stage deferred normalization of the previous block:
            # stage 1 = den copy + reciprocal (DVE) + broadcast (GpSimd),
            # stage 2 = normalize multiplies (DVE, reading P@V PSUM directly)
            pending = [None, None]

            def attention(j, p, fillers, last=False):
                # software-pipelined: QK(g+1) issues before P@V(g); one
                # filler (proj / out-proj chunk) per group keeps PE dense
                # while ACT drains each group's score tiles through exp
                nkb = 4 * (j + 1)       # causal: k-blocks 0..nkb-1
                ngr = nkb // 2
                qsl = slice(j * 512, (j + 1) * 512)
                fillers = list(fillers)

                def chain(s, f):
                    if s is None:
                        return f
                    if f is None:
                        return s
                    return lambda s=s, f=f: (s(), f())
                fillers[0] = chain(pending[0], fillers[0])
                fillers[1] = chain(pending[1], fillers[1])
                pending[0] = pending[1] = None
                pv = [psp.tile([65, 512], F32, tag="pv", name=f"pv{_hh}")
                      for _hh in range(2)]
                st_g = {}

                def qk_group(g):
                    st = [psp.tile([128, 1024], F32, tag="st",
                                   name=f"st{_hh}") for _hh in range(2)]
                    for i in range(2):
                        kb = 2 * g + i
                        # fully-masked cols [0, rel) are never consumed (the
                        # P@V moving slice skips them) — don't compute them
                        lo = max(kb * 128 - j * 512, 0)
                        for hh in range(2):  # packed rows 0-63/64-127
                            oh = hh * 64
                            nc.tensor.matmul(
                                st[hh][:, i * 512 + lo:(i + 1) * 512],
                                (kt[oh:oh + 64, p, kb * 128:(kb + 1) * 128]),
                                (qt[oh:oh + 64, p,
                                    j * 512 + lo:(j + 1) * 512]),
                                start=True, stop=True)
                    st_g[g] = st

                qk_group(0)
                for g in range(ngr):
                    st = st_g.pop(g)
                    # mask the diagonal 128x128 triangles with -1e9 (exp -> 0)
                    r0 = max(2 * g * 128 - j * 512, 0)
                    r1 = max((2 * g + 1) * 128 - j * 512, 0)
                    for i, rr in ((0, 2 * g * 128 - j * 512),
                                  (1, (2 * g + 1) * 128 - j * 512)):
                        if rr >= 0:
                            for hh in range(2):
                                sl = st[hh][:, i * 512 + rr:
                                            i * 512 + rr + 128]
                                nc.vector.tensor_add(sl, sl, tri_t)
                    pt = [None, None]
                    for hh in range(2):
                        pt[hh] = sm.tile([128, 1024], BF16, tag="pt",
                                         name=f"pt{hh}", bufs=6)
                        # exp only the causally-valid column spans
                        if r1 <= 192:
                            nc.scalar.activation(pt[hh][:, r0:1024],
                                                 st[hh][:, r0:1024], AF.Exp)
                        else:
                            nc.scalar.activation(pt[hh][:, r0:512],
                                                 st[hh][:, r0:512], AF.Exp)
                            nc.scalar.activation(
                                pt[hh][:, 512 + r1:1024],
                                st[hh][:, 512 + r1:1024], AF.Exp)
                    if g + 1 < ngr:
                        qk_group(g + 1)
                    if fillers:
                        f = fillers.pop(0)
                        if f is not None:
                            f()
                    for i in range(2):
                        kb = 2 * g + i
                        rel = max(kb * 128 - j * 512, 0)
                        for hh in range(2):
                            h = 2 * p + hh
                            nc.tensor.matmul(
                                pv[hh][:, rel:512],
                                (va[:, kb, h * 65:h * 65 + 65]),
                                (pt[hh][:, i * 512 + rel:(i + 1) * 512]),
                                start=(kb == 0), stop=(kb == nkb - 1),
                                skip_group_check=True)
                bcs = [None]

                def _stage1(pv=pv, bcs=bcs):
                    # both heads' denominator rows into one [1,1024] tile:
                    # a single reciprocal and a single GpSimd broadcast
                    # (the DSP launch overhead dominates the broadcast)
                    den = sm.tile([1, 1024], F32, tag="den")
                    rec = sm.tile([1, 1024], F32, tag="rec", name="rec")
                    for hh in range(2):
                        nc.vector.tensor_copy(den[:, hh * 512:hh * 512 + 512],
                                              pv[hh][64:65, :])
                    nc.vector.reciprocal_approx_fast(rec, den)
                    bcs[0] = sm.tile([64, 1024], F32, tag="bcs", name="bcs")
                    nc.gpsimd.partition_broadcast(bcs[0], rec)

                def _stage2(pv=pv, bcs=bcs, p=p, qsl=qsl):
                    for hh in range(2):
                        nc.vector.tensor_mul(
                            otn[hh * 64:hh * 64 + 64, p, qsl],
                            pv[hh][0:64, :],
                            bcs[0][:, hh * 512:hh * 512 + 512])
                if last:
                    # start the final norm chain at P@V completion, before
                    # the drained fillers (whose PE work hides its latency)
                    _stage1()
                for f in fillers:   # drain unused slots
                    if f is not None:
                        f()
                if last:
                    return _stage2
                pending[0], pending[1] = _stage1, _stage2

            P = qk_proj
            V = v_proj
            O = out_tile
            # ---- prologue: minimum inputs for attention(0, 0) ------------
            qk_proj00()
            # ---- head-pair 0: V tiles + remaining projections as fillers -
            attention(0, 0, [lambda: (V(0), V(1)),
                             lambda: (V(2), V(3),
                                      P(wq_t, qt, bq_t, 0, 1)),
                             lambda: P(wk_t, kt, bk_t, 0, 1)])
            attention(1, 0, [lambda: (V(4), V(5)),
                             lambda: V(6), lambda: V(7),
                             lambda: P(wq_t, qt, bq_t, 0, 2),
                             lambda: P(wk_t, kt, bk_t, 0, 2)])
            attention(2, 0, [lambda: V(8), lambda: V(9),
                             lambda: V(10), lambda: V(11),
                             lambda: P(wq_t, qt, bq_t, 0, 3),
                             lambda: P(wk_t, kt, bk_t, 0, 3)])
            attention(3, 0, [lambda: V(12), lambda: V(13),
                             lambda: V(14), lambda: V(15),
                             lambda: P(wq_t, qt, bq_t, 1, 0),
                             lambda: P(wk_t, kt, bk_t, 1, 0),
                             lambda: P(wq_t, qt, bq_t, 1, 1),
                             lambda: P(wk_t, kt, bk_t, 1, 1)])
            # ---- head-pair 1, with out-projection chunks as fillers ------
            attention(0, 1, [lambda: P(wq_t, qt, bq_t, 1, 2),
                             lambda: P(wk_t, kt, bk_t, 1, 2)])
            attention(1, 1, [lambda: P(wq_t, qt, bq_t, 1, 3),
                             lambda: P(wk_t, kt, bk_t, 1, 3),
                             lambda: O(0, 0), lambda: O(0, 1)])
            attention(2, 1, [lambda: O(0, 2), lambda: O(0, 3),
                             lambda: O(1, 4, (0,)), lambda: O(1, 4, (1,)),
                             lambda: O(1, 5, (0,)), lambda: O(1, 5, (1,))])
            norm_last = attention(
                3, 1, [lambda: O(1, 6, (0,)), lambda: O(1, 6, (1,)),
                       lambda: O(1, 7, (0,)), lambda: O(1, 7, (1,)),
                       lambda: O(2, 8), lambda: O(2, 9),
                       None, None,
                       lambda: O(2, 10), lambda: O(2, 11)],  # drain: under chain
                last=True)
            # ---- tail: PE works under the final norm chain ---------------
            pst12 = out_tail_c0(12)
            pst13 = out_tail_c0(13)
            ps14 = [psp.tile([128, 512], F32, tag="ps", name="ops")
                    for _ in range(2)]
            for n in range(2):
                nc.tensor.matmul(ps14[n], (otn[:, 0, 14 * 128:15 * 128]),
                                 (wo_t[:, 0, n * 512:(n + 1) * 512]),
                                 start=True, stop=False)
            norm_last()
            out_tile_tail(12, pst12)
            out_tile_tail(13, pst13)
            so14 = sm.tile([128, 1024], BF16, tag="so", name="so", bufs=3)
            for n in range(2):
                nc.tensor.matmul(ps14[n], (otn[:, 1, 14 * 128:15 * 128]),
                                 (wo_t[:, 1, n * 512:(n + 1) * 512]),
                                 start=False, stop=True)
                nc.vector.tensor_copy(so14[:, n * 512:(n + 1) * 512],
                                      ps14[n])
            nc.sync.dma_start(out[14 * 128:15 * 128, :], so14)
            pst15 = out_tail_c0(15)
            out_tile_tail(15, pst15)

    nc.compile()
    return nc


def _tri_np():
    # within-tile causal triangle: tri[kk, c] = NEG if c < kk else 0
    cs = np.arange(128)[None, :]
    ks = np.arange(128)[:, None]
    return np.where(cs < ks, np.float32(NEG),
                    np.float32(0.0)).astype(np.float32)


def build_in_maps(x, Wq, bq, Wk, bk, Wv, bv, Wo):
    tri_np = _tri_np()
    onesf_np = np.ones((1, 64), dtype=np.float32)
    xT_b = [np.asarray(x[b].T, dtype=np.float32).astype(bfloat16)
            for b in range(B)]
    Wq8 = (np.asarray(Wq, dtype=np.float32) * 0.125)  # fold 1/sqrt(DH) into Q
    in_maps = []
    for c in range(N_CORES):
        b, tp = divmod(c, TPG)
        sl = slice(tp * CH, (tp + 1) * CH)
        wv_aug = np.zeros((D, CHA), dtype=np.float32)
        bv_aug = np.zeros((1, CHA), dtype=np.float32)
        for h in range(HPC):
            hsl = slice(tp * CH + h * DH, tp * CH + (h + 1) * DH)
            wv_aug[:, h * 65:h * 65 + DH] = Wv[:, hsl]
            bv_aug[0, h * 65:h * 65 + DH] = bv[hsl]
            bv_aug[0, h * 65 + DH] = 1.0
        in_maps.append({
            "xT": xT_b[b],
            "wq": Wq8[:, sl].astype(bfloat16),
            "wk": np.asarray(Wk[:, sl], dtype=np.float32).astype(bfloat16),
            "wv": wv_aug.astype(bfloat16),
            "wo": np.asarray(Wo[sl, :], dtype=np.float32).astype(bfloat16),
            "bq": (bq[sl].astype(np.float32) * 0.125).reshape(2, 128).T.copy(),
            "bk": bk[sl].astype(np.float32).reshape(2, 128).T.copy(),
            "bv": bv_aug.astype(bfloat16),
            "tri": tri_np,
            "onesf": onesf_np,
        })
    return in_maps


def _get_program():
    global _PROG
    if _PROG is None:
        _PROG = _build_program()
    return _PROG


def kernel(x, mask, Wq, bq, Wk, bk, Wv, bv, Wo, bo):
    x = np.asarray(x, dtype=np.float32)
    mask = np.asarray(mask)
    Wq, Wk, Wv, Wo = (np.asarray(w, dtype=np.float32)
                      for w in (Wq, Wk, Wv, Wo))
    bq, bk, bv, bo = (np.asarray(b, dtype=np.float32)
                      for b in (bq, bk, bv, bo))
    causal = bool(
        np.array_equal(mask != 0,
                       np.tril(np.ones((S, S), dtype=bool))))
    if not causal:
        # Fallback for non-causal masks: exact host computation.
        q = (x @ Wq + bq).reshape(B, S, H, DH).transpose(0, 2, 1, 3)
        k = (x @ Wk + bk).reshape(B, S, H, DH).transpose(0, 2, 1, 3)
        v = (x @ Wv + bv).reshape(B, S, H, DH).transpose(0, 2, 1, 3)
        attn = np.einsum("bhqd,bhkd->bhqk", q, k) / np.sqrt(np.float32(DH))
        attn = np.where(mask == 0, np.float32(-1e9), attn)
        attn = attn - attn.max(axis=-1, keepdims=True)
        e = np.exp(attn)
        p = e / e.sum(axis=-1, keepdims=True)
        o = np.einsum("bhqk,bhkd->bhqd", p, v)
        o = o.transpose(0, 2, 1, 3).reshape(B, S, D)
        return (o @ Wo + bo).astype(np.float32)

    nc = _get_program()
    in_maps = build_in_maps(x, Wq, bq, Wk, bk, Wv, bv, Wo)
    res = run_bass_kernel_spmd(nc, in_maps, core_ids=list(range(N_CORES)))
    out = np.zeros((B, S, D), dtype=np.float32)
    for c in range(N_CORES):
        out[c // TPG] += res.results[c]["out"].astype(np.float32)
    out += bo.astype(np.float32)
    return out
